# revision 8
# baseline (speedup 1.0000x reference)
"""nn_MergeWindows — Trainium2 Bass kernel (8 NeuronCores, SPMD over image rows).

The reference computes out[b,c,y,x] = 1.0 iff remap[argmax_d masks[b,d,y,x]] == c,
where remap: [32]->[32] merges channels according to a sequential scan over
window-adjacency candidate pairs.  remap depends only on tiny metadata (cosine
sims of the [4,7,64] slot features and per-channel edge-touch bits along the
four window boundary strips) and is computed on the host in microseconds.

Device strategy (per core: 128 image rows, [32, 128, 1024] slab, slot-ordered):

  Channels are permuted host-side into slots [keepers | unmerged | rems], where
  a "keeper" receives >=1 merged channel.  Since out[c] = 1 iff the max over
  c's merge group equals the pixel max, the device:
    1. max-accumulates each rem slot into its keeper slot (merge "rounds" —
       one strided tensor_tensor max per round over all keepers needing it),
    2. tree-maxes the NREAL real slots -> per-pixel max mx,
    3. emits the one-hot with two is_equal ops against broadcast mx,
    4. detects argmax TIES (pixels where >=2 real slots equal mx) via an ACT
       Identity op with accum_out: per-partition sum over the one-hot == G
       unless a tie occurred.  Flagged rows (a handful per image at most) are
       recomputed exactly on the host, making the kernel exact for all inputs.
  Merged-away channels are identically zero: the device only writes the NREAL
  live channels (9.5 MiB instead of 16 MiB per core) and the host zero-fills
  the rest.
"""

import json

import numpy as np

N_WINDOWS = 4
WIN_H = WIN_W = 512
IMG_H = IMG_W = 1024
C = 32
MPW = C // N_WINDOWS
SIM_THRESH = 0.1

N_CORES = 8
ROWS_PER_CORE = IMG_H // N_CORES  # 128
# column-tile widths: narrow first/last tiles shorten the pipeline ramp
# (compute starts after tile 0 lands) and the drain tail
WIDTHS = (128, 256, 256, 256, 128)
NTILES = len(WIDTHS)
assert sum(WIDTHS) == IMG_W

MERGE_INPLACE = True     # accumulate rounds in place on the keeper tile
DET_ON_ACT = True        # tie detector via scalar-engine accum_out

_cache = {}


# --------------------------------------------------------------------------
# host-side merge decision (mirrors reference._merge_windows metadata math)
# --------------------------------------------------------------------------
def _compute_remap(masks, slot_features, pl, pt):
    B, Ch, H, W = masks.shape
    mpw = Ch // N_WINDOWS
    ranges = [(i * mpw, (i + 1) * mpw) for i in range(N_WINDOWS)]

    adjacency = []
    for i in range(N_WINDOWS):
        for j in range(i + 1, N_WINDOWS):
            if pt[i] == pt[j] and abs(pl[i] - pl[j]) == WIN_W:
                adjacency.append((i, j, True) if pl[i] < pl[j] else (j, i, True))
            if pl[i] == pl[j] and abs(pt[i] - pt[j]) == WIN_H:
                adjacency.append((i, j, False) if pt[i] < pt[j] else (j, i, False))

    edge_l = np.zeros(Ch, bool)
    edge_r = np.zeros(Ch, bool)
    edge_t = np.zeros(Ch, bool)
    edge_b = np.zeros(Ch, bool)
    m0 = masks[0]
    for wi, (s, e) in enumerate(ranges):
        ys, ye = max(pt[wi], 0), min(pt[wi] + WIN_H, H)
        xs, xe = max(pl[wi], 0), min(pl[wi] + WIN_W, W)
        if ys >= ye or xs >= xe:
            continue
        ids_l = np.argmax(m0[:, ys:ye, xs], axis=0)
        ids_r = np.argmax(m0[:, ys:ye, xe - 1], axis=0)
        ids_t = np.argmax(m0[:, ys, xs:xe], axis=0)
        ids_b = np.argmax(m0[:, ye - 1, xs:xe], axis=0)
        for k in range(s, e):
            edge_l[k] = np.any(ids_l == k)
            edge_r[k] = np.any(ids_r == k)
            edge_t[k] = np.any(ids_t == k)
            edge_b[k] = np.any(ids_b == k)

    ci_l, cj_l, wi_l, wj_l, hz_l = [], [], [], [], []
    for wi, wj, horiz in adjacency:
        si, ei = ranges[wi]
        sj, ej = ranges[wj]
        for ci in range(si + 1, ei):
            for cj in range(sj + 1, ej):
                ci_l.append(ci)
                cj_l.append(cj)
                wi_l.append(wi)
                wj_l.append(wj)
                hz_l.append(horiz)

    target = np.arange(Ch)
    if not ci_l:
        return target

    sf = np.asarray(slot_features, np.float32)
    sf_n = sf / (np.linalg.norm(sf, axis=-1, keepdims=True) + np.float32(1e-8))
    ci_a = np.array(ci_l)
    cj_a = np.array(cj_l)
    rel_i = ci_a % mpw - 1
    rel_j = cj_a % mpw - 1
    fi = sf_n[np.array(wi_l), rel_i]
    fj = sf_n[np.array(wj_l), rel_j]
    sims = np.sum(fi * fj, axis=-1)
    hz = np.array(hz_l)
    edge_ok = np.where(hz, edge_r[ci_a] & edge_l[cj_a], edge_b[ci_a] & edge_t[cj_a])
    passing = edge_ok & (sims > np.float32(SIM_THRESH))

    merged = np.zeros(Ch, bool)
    for ci, cj, ok in zip(ci_l, cj_l, passing):
        if ok and not merged[ci] and not merged[cj]:
            keep, rem = min(ci, cj), max(ci, cj)
            target[target == rem] = keep
            merged[rem] = True
    return target


def _slot_layout(remap):
    """Channel->slot permutation and merge-round structure from remap.

    Returns (slot_order, real_channels, nk, round_sizes) where slot_order[s]
    is the original channel loaded into slot s; slots [0:nk] keepers sorted by
    descending rem count, [nk:nreal] unmerged, [nreal:32] rems grouped so that
    round r's rems (the r-th rem of each keeper that has one) are contiguous
    and aligned with keeper slots [0:round_sizes[r]].
    """
    rems_of = {}
    for c in range(C):
        k = int(remap[c])
        if k != c:
            rems_of.setdefault(k, []).append(c)
    keepers = sorted(rems_of, key=lambda k: (-len(rems_of[k]), k))
    unmerged = [c for c in range(C)
                if int(remap[c]) == c and c not in rems_of]
    nk = len(keepers)
    nreal = nk + len(unmerged)
    max_rounds = max((len(v) for v in rems_of.values()), default=0)
    round_slots = []
    round_sizes = []
    for r in range(max_rounds):
        rs = [rems_of[k][r] for k in keepers if len(rems_of[k]) > r]
        round_sizes.append(len(rs))
        round_slots.extend(rs)
    slot_order = keepers + unmerged + round_slots
    assert len(slot_order) == C
    return slot_order, keepers + unmerged, nk, tuple(round_sizes)


# --------------------------------------------------------------------------
# wait-split post-pass: the pinned neuronxcc allows only ONE sync wait per
# instruction; hoist extras onto preceding same-engine EventSemaphore insts.
# --------------------------------------------------------------------------
def _split_excess_waits(bir_json_bytes, limit=1):
    j = json.loads(bir_json_bytes)
    counter = [0]
    for fn in j.get("functions", []):
        for bb in fn.get("blocks", []):
            new_insts = []
            for inst in bb.get("instructions", []):
                si = inst.get("sync_info") or {}
                waits = si.get("on_wait") or []
                if len(waits) > limit:
                    extra = waits[: len(waits) - limit]
                    si["on_wait"] = waits[len(waits) - limit:]
                    inst["sync_info"] = si
                    for i in range(0, len(extra), limit):
                        counter[0] += 1
                        new_insts.append({
                            "engine": inst["engine"],
                            "ins": [],
                            "name": f"{inst['name']}_hoistw{counter[0]}",
                            "opcode": "EventSemaphore",
                            "outs": [],
                            "sync_info": {"on_update": [],
                                          "on_wait": extra[i: i + limit]},
                        })
                new_insts.append(inst)
            bb["instructions"] = new_insts
    return json.dumps(j).encode()


def _bcast_mid(ap, n):
    """Broadcast a [128, 1, G]-ish AP over n slots (middle dim, stride 0)."""
    import concourse.bass as bass
    return bass.AP(tensor=ap.tensor, offset=ap.offset,
                   ap=[ap.ap[0], [0, n], ap.ap[-1]])


def _build_program(nk, round_sizes, nreal):
    key = ("prog", nk, round_sizes, nreal)
    if key in _cache:
        return _cache[key]

    import concourse.bass as bass
    import concourse.tile as tile
    from concourse import mybir

    f32 = mybir.dt.float32
    f16 = mybir.dt.float16
    MAX = mybir.AluOpType.max
    EQ = mybir.AluOpType.is_equal

    nu = nreal - nk          # unmerged slot count

    nc = bass.Bass()
    masks_in = nc.dram_tensor("masks", [C, ROWS_PER_CORE, IMG_W], f32,
                              kind="ExternalInput")
    out_dram = nc.dram_tensor("out", [nreal, ROWS_PER_CORE, IMG_W], f32,
                              kind="ExternalOutput")
    acc_dram = nc.dram_tensor("acc", [128, NTILES], f32,
                              kind="ExternalOutput")

    with tile.TileContext(nc) as tc:
        with (
            tc.tile_pool(name="inp", bufs=3) as inp,
            tc.tile_pool(name="outp", bufs=2) as outp,
            tc.tile_pool(name="work", bufs=1) as work,
            tc.tile_pool(name="singles", bufs=1) as singles,
        ):
            acc = singles.tile([128, NTILES], f32)

            WMAX = max(WIDTHS)
            col = 0
            for t in range(NTILES):
                G = WIDTHS[t]
                sl = slice(col, col + G)
                col += G
                in_f = inp.tile([128, C, WMAX], f32, tag="in_t")
                in_t = in_f[:, :, 0:G]
                nc.sync.dma_start(
                    in_t, masks_in[:, :, sl].rearrange("d p g -> p d g"))

                # ---- merge rounds: accumulate rems into keeper slots ----
                if nk:
                    kf_f = work.tile([128, nk, WMAX], f32, tag="kf")
                    kf = kf_f[:, :, 0:G]
                    off = nreal
                    for r, nr in enumerate(round_sizes):
                        if r == 0:
                            nc.vector.tensor_tensor(
                                out=kf[:], in0=in_t[:, 0:nk, :],
                                in1=in_t[:, off:off + nr, :], op=MAX)
                        else:
                            nc.vector.tensor_tensor(
                                out=kf[:, 0:nr, :], in0=kf[:, 0:nr, :],
                                in1=in_t[:, off:off + nr, :], op=MAX)
                        off += nr

                # ---- tree max over the nreal real slots -> mx [128,1,G] ----
                # segments: (tile, start, count) pieces holding real values
                segments = []
                if nk:
                    segments.append((kf, 0, nk))
                if nu:
                    segments.append((in_t, nk, nu))
                level = 0
                total = sum(c for (_, _, c) in segments)
                while total > 1:
                    dest_f = work.tile([128, (total + 1) // 2, WMAX], f32,
                                       tag=f"tree{level}")
                    dest = dest_f[:, :, 0:G]
                    pos = 0
                    singles = []
                    for (tl, st, cnt) in segments:
                        h = cnt // 2
                        if h:
                            nc.vector.tensor_tensor(
                                out=dest[:, pos:pos + h, :],
                                in0=tl[:, st:st + h, :],
                                in1=tl[:, st + h:st + 2 * h, :], op=MAX)
                            pos += h
                        if cnt % 2:
                            singles.append((tl, st + 2 * h))
                    while len(singles) >= 2:
                        (t0, s0), (t1, s1) = singles.pop(), singles.pop()
                        nc.vector.tensor_tensor(
                            out=dest[:, pos:pos + 1, :],
                            in0=t0[:, s0:s0 + 1, :],
                            in1=t1[:, s1:s1 + 1, :], op=MAX)
                        pos += 1
                    segments = [(dest, 0, pos)] + \
                        [(tl, st, 1) for (tl, st) in singles]
                    total = sum(c for (_, _, c) in segments)
                    level += 1
                mxt, mxs, _ = segments[0]
                mx_ap = mxt[:, mxs:mxs + 1, :]

                # ---- one-hot via is_equal against broadcast mx ----
                out_f = outp.tile([128, nreal, WMAX], f32, tag="out_t")
                out_t = out_f[:, :, 0:G]
                if nk:
                    nc.vector.tensor_tensor(
                        out=out_t[:, 0:nk, :], in0=kf[:],
                        in1=_bcast_mid(mx_ap, nk), op=EQ)
                if nu:
                    nc.vector.tensor_tensor(
                        out=out_t[:, nk:nreal, :], in0=in_t[:, nk:nreal, :],
                        in1=_bcast_mid(mx_ap, nu), op=EQ)

                # ---- tie detector: per-partition popcount of the one-hot ----
                if DET_ON_ACT:
                    dummy = work.tile([128, nreal, WMAX], f16, tag="dummy")
                    nc.scalar.activation(
                        dummy[:, :, 0:G], out_t,
                        mybir.ActivationFunctionType.Identity,
                        accum_out=acc[:, t:t + 1])
                else:
                    nc.vector.tensor_reduce(
                        out=acc[:, t:t + 1], in_=out_t[:],
                        axis=mybir.AxisListType.XY, op=mybir.AluOpType.add)

                nc.sync.dma_start(
                    out_dram[:, :, sl].rearrange("c p g -> p c g"), out_t[:])

            nc.sync.dma_start(acc_dram[:], acc[:])

    orig = nc.to_json_bytes
    nc.to_json_bytes = lambda: _split_excess_waits(orig())
    _cache[key] = nc
    return nc


def kernel(masks, slot_features, pad_left, pad_top):
    from concourse.bass_utils import run_bass_kernel_spmd

    masks = np.asarray(masks, np.float32)
    slot_features = np.asarray(slot_features, np.float32)
    pl = [int(v) for v in np.asarray(pad_left)]
    pt = [int(v) for v in np.asarray(pad_top)]

    remap = _compute_remap(masks, slot_features, pl, pt)
    slot_order, real_channels, nk, round_sizes = _slot_layout(remap)
    nreal = len(real_channels)

    nc = _build_program(nk, round_sizes, nreal)

    m0 = masks[0][slot_order]           # [32, H, W] slot-ordered
    in_maps = []
    for i in range(N_CORES):
        slab = np.ascontiguousarray(
            m0[:, i * ROWS_PER_CORE:(i + 1) * ROWS_PER_CORE, :])
        in_maps.append({"masks": slab})

    res = run_bass_kernel_spmd(nc, in_maps, core_ids=list(range(N_CORES)))

    out = np.zeros((1, C, IMG_H, IMG_W), np.float32)
    real_idx = np.asarray(real_channels)
    flagged_rows = []
    for i, r in enumerate(res.results):
        out[0, real_idx, i * ROWS_PER_CORE:(i + 1) * ROWS_PER_CORE, :] = \
            r["out"]
        acc = r["acc"]                  # [128, NTILES]
        bad = np.where(acc.sum(axis=1) != float(IMG_W))[0]
        for p in bad:
            flagged_rows.append(i * ROWS_PER_CORE + int(p))

    # exact host patch of rows containing argmax ties (typically <= 3 rows)
    for y in flagged_rows:
        w = np.argmax(masks[0, :, y, :], axis=0)          # first max, like jnp
        out[0, :, y, :] = 0.0
        out[0, remap[w], y, np.arange(IMG_W)] = 1.0
    return out


# revision 9
# speedup vs baseline: 1.0800x; 1.0800x over previous
"""nn_MergeWindows — Trainium2 Bass kernel (8 NeuronCores, SPMD over image rows).

The reference computes out[b,c,y,x] = 1.0 iff remap[argmax_d masks[b,d,y,x]] == c,
where remap: [32]->[32] merges channels according to a sequential scan over
window-adjacency candidate pairs.  remap depends only on tiny metadata (cosine
sims of the [4,7,64] slot features and per-channel edge-touch bits along the
four window boundary strips) and is computed on the host in microseconds.

Device strategy (per core: 128 image rows, [32, 128, 1024] slab, slot-ordered):

  Channels are permuted host-side into slots [keepers | unmerged | rems], where
  a "keeper" receives >=1 merged channel.  Since out[c] = 1 iff the max over
  c's merge group equals the pixel max, the device:
    1. max-accumulates each rem slot into its keeper slot (merge "rounds" —
       one strided tensor_tensor max per round over all keepers needing it),
    2. tree-maxes the NREAL real slots -> per-pixel max mx,
    3. emits the one-hot with two is_equal ops against broadcast mx,
    4. detects argmax TIES (pixels where >=2 real slots equal mx) via an ACT
       Identity op with accum_out: per-partition sum over the one-hot == G
       unless a tie occurred.  Flagged rows (a handful per image at most) are
       recomputed exactly on the host, making the kernel exact for all inputs.
  Merged-away channels are identically zero: the device only writes the NREAL
  live channels (9.5 MiB instead of 16 MiB per core) and the host zero-fills
  the rest.
"""

import json

import numpy as np

N_WINDOWS = 4
WIN_H = WIN_W = 512
IMG_H = IMG_W = 1024
C = 32
MPW = C // N_WINDOWS
SIM_THRESH = 0.1

N_CORES = 8
ROWS_PER_CORE = IMG_H // N_CORES  # 128
# column-tile widths: narrow first/last tiles shorten the pipeline ramp
# (compute starts after tile 0 lands) and the drain tail
WIDTHS = (256, 256, 256, 256)
NTILES = len(WIDTHS)
assert sum(WIDTHS) == IMG_W

MERGE_INPLACE = True     # accumulate rounds in place on the keeper tile
DET_ON_ACT = True        # tie detector via scalar-engine accum_out

_cache = {}


# --------------------------------------------------------------------------
# host-side merge decision (mirrors reference._merge_windows metadata math)
# --------------------------------------------------------------------------
def _compute_remap(masks, slot_features, pl, pt):
    B, Ch, H, W = masks.shape
    mpw = Ch // N_WINDOWS
    ranges = [(i * mpw, (i + 1) * mpw) for i in range(N_WINDOWS)]

    adjacency = []
    for i in range(N_WINDOWS):
        for j in range(i + 1, N_WINDOWS):
            if pt[i] == pt[j] and abs(pl[i] - pl[j]) == WIN_W:
                adjacency.append((i, j, True) if pl[i] < pl[j] else (j, i, True))
            if pl[i] == pl[j] and abs(pt[i] - pt[j]) == WIN_H:
                adjacency.append((i, j, False) if pt[i] < pt[j] else (j, i, False))

    edge_l = np.zeros(Ch, bool)
    edge_r = np.zeros(Ch, bool)
    edge_t = np.zeros(Ch, bool)
    edge_b = np.zeros(Ch, bool)
    m0 = masks[0]
    for wi, (s, e) in enumerate(ranges):
        ys, ye = max(pt[wi], 0), min(pt[wi] + WIN_H, H)
        xs, xe = max(pl[wi], 0), min(pl[wi] + WIN_W, W)
        if ys >= ye or xs >= xe:
            continue
        ids_l = np.argmax(m0[:, ys:ye, xs], axis=0)
        ids_r = np.argmax(m0[:, ys:ye, xe - 1], axis=0)
        ids_t = np.argmax(m0[:, ys, xs:xe], axis=0)
        ids_b = np.argmax(m0[:, ye - 1, xs:xe], axis=0)
        for k in range(s, e):
            edge_l[k] = np.any(ids_l == k)
            edge_r[k] = np.any(ids_r == k)
            edge_t[k] = np.any(ids_t == k)
            edge_b[k] = np.any(ids_b == k)

    ci_l, cj_l, wi_l, wj_l, hz_l = [], [], [], [], []
    for wi, wj, horiz in adjacency:
        si, ei = ranges[wi]
        sj, ej = ranges[wj]
        for ci in range(si + 1, ei):
            for cj in range(sj + 1, ej):
                ci_l.append(ci)
                cj_l.append(cj)
                wi_l.append(wi)
                wj_l.append(wj)
                hz_l.append(horiz)

    target = np.arange(Ch)
    if not ci_l:
        return target

    sf = np.asarray(slot_features, np.float32)
    sf_n = sf / (np.linalg.norm(sf, axis=-1, keepdims=True) + np.float32(1e-8))
    ci_a = np.array(ci_l)
    cj_a = np.array(cj_l)
    rel_i = ci_a % mpw - 1
    rel_j = cj_a % mpw - 1
    fi = sf_n[np.array(wi_l), rel_i]
    fj = sf_n[np.array(wj_l), rel_j]
    sims = np.sum(fi * fj, axis=-1)
    hz = np.array(hz_l)
    edge_ok = np.where(hz, edge_r[ci_a] & edge_l[cj_a], edge_b[ci_a] & edge_t[cj_a])
    passing = edge_ok & (sims > np.float32(SIM_THRESH))

    merged = np.zeros(Ch, bool)
    for ci, cj, ok in zip(ci_l, cj_l, passing):
        if ok and not merged[ci] and not merged[cj]:
            keep, rem = min(ci, cj), max(ci, cj)
            target[target == rem] = keep
            merged[rem] = True
    return target


def _slot_layout(remap):
    """Channel->slot permutation and merge-round structure from remap.

    Returns (slot_order, real_channels, nk, round_sizes) where slot_order[s]
    is the original channel loaded into slot s; slots [0:nk] keepers sorted by
    descending rem count, [nk:nreal] unmerged, [nreal:32] rems grouped so that
    round r's rems (the r-th rem of each keeper that has one) are contiguous
    and aligned with keeper slots [0:round_sizes[r]].
    """
    rems_of = {}
    for c in range(C):
        k = int(remap[c])
        if k != c:
            rems_of.setdefault(k, []).append(c)
    keepers = sorted(rems_of, key=lambda k: (-len(rems_of[k]), k))
    unmerged = [c for c in range(C)
                if int(remap[c]) == c and c not in rems_of]
    nk = len(keepers)
    nreal = nk + len(unmerged)
    max_rounds = max((len(v) for v in rems_of.values()), default=0)
    round_slots = []
    round_sizes = []
    for r in range(max_rounds):
        rs = [rems_of[k][r] for k in keepers if len(rems_of[k]) > r]
        round_sizes.append(len(rs))
        round_slots.extend(rs)
    slot_order = keepers + unmerged + round_slots
    assert len(slot_order) == C
    return slot_order, keepers + unmerged, nk, tuple(round_sizes)


# --------------------------------------------------------------------------
# wait-split post-pass: the pinned neuronxcc allows only ONE sync wait per
# instruction; hoist extras onto preceding same-engine EventSemaphore insts.
# --------------------------------------------------------------------------
def _split_excess_waits(bir_json_bytes, limit=1):
    j = json.loads(bir_json_bytes)
    counter = [0]
    for fn in j.get("functions", []):
        for bb in fn.get("blocks", []):
            new_insts = []
            for inst in bb.get("instructions", []):
                si = inst.get("sync_info") or {}
                waits = si.get("on_wait") or []
                if len(waits) > limit:
                    extra = waits[: len(waits) - limit]
                    si["on_wait"] = waits[len(waits) - limit:]
                    inst["sync_info"] = si
                    for i in range(0, len(extra), limit):
                        counter[0] += 1
                        new_insts.append({
                            "engine": inst["engine"],
                            "ins": [],
                            "name": f"{inst['name']}_hoistw{counter[0]}",
                            "opcode": "EventSemaphore",
                            "outs": [],
                            "sync_info": {"on_update": [],
                                          "on_wait": extra[i: i + limit]},
                        })
                new_insts.append(inst)
            bb["instructions"] = new_insts
    return json.dumps(j).encode()


def _bcast_mid(ap, n):
    """Broadcast a [128, 1, G]-ish AP over n slots (middle dim, stride 0)."""
    import concourse.bass as bass
    return bass.AP(tensor=ap.tensor, offset=ap.offset,
                   ap=[ap.ap[0], [0, n], ap.ap[-1]])


def _build_program(nk, round_sizes, nreal):
    key = ("prog", nk, round_sizes, nreal)
    if key in _cache:
        return _cache[key]

    import concourse.bass as bass
    import concourse.tile as tile
    from concourse import mybir

    f32 = mybir.dt.float32
    f16 = mybir.dt.float16
    MAX = mybir.AluOpType.max
    EQ = mybir.AluOpType.is_equal

    nu = nreal - nk          # unmerged slot count

    nc = bass.Bass()
    masks_in = nc.dram_tensor("masks", [C, ROWS_PER_CORE, IMG_W], f32,
                              kind="ExternalInput")
    out_dram = nc.dram_tensor("out", [nreal, ROWS_PER_CORE, IMG_W], f32,
                              kind="ExternalOutput")
    acc_dram = nc.dram_tensor("acc", [128, NTILES], f32,
                              kind="ExternalOutput")

    with tile.TileContext(nc) as tc:
        with (
            tc.tile_pool(name="inp", bufs=3) as inp,
            tc.tile_pool(name="outp", bufs=2) as outp,
            tc.tile_pool(name="work", bufs=1) as work,
            tc.tile_pool(name="singles", bufs=1) as singles,
        ):
            acc = singles.tile([128, NTILES], f32)

            WMAX = max(WIDTHS)
            col = 0
            for t in range(NTILES):
                G = WIDTHS[t]
                sl = slice(col, col + G)
                col += G
                in_f = inp.tile([128, C, WMAX], f32, tag="in_t")
                in_t = in_f[:, :, 0:G]
                nc.sync.dma_start(
                    in_t, masks_in[:, :, sl].rearrange("d p g -> p d g"))

                # ---- merge rounds: accumulate rems into keeper slots ----
                if nk:
                    kf_f = work.tile([128, nk, WMAX], f32, tag="kf")
                    kf = kf_f[:, :, 0:G]
                    off = nreal
                    for r, nr in enumerate(round_sizes):
                        if r == 0:
                            nc.vector.tensor_tensor(
                                out=kf[:], in0=in_t[:, 0:nk, :],
                                in1=in_t[:, off:off + nr, :], op=MAX)
                        else:
                            nc.vector.tensor_tensor(
                                out=kf[:, 0:nr, :], in0=kf[:, 0:nr, :],
                                in1=in_t[:, off:off + nr, :], op=MAX)
                        off += nr

                # ---- tree max over the nreal real slots -> mx [128,1,G] ----
                # segments: (tile, start, count) pieces holding real values
                segments = []
                if nk:
                    segments.append((kf, 0, nk))
                if nu:
                    segments.append((in_t, nk, nu))
                level = 0
                total = sum(c for (_, _, c) in segments)
                while total > 1:
                    dest_f = work.tile([128, (total + 1) // 2, WMAX], f32,
                                       tag=f"tree{level}")
                    dest = dest_f[:, :, 0:G]
                    pos = 0
                    singles = []
                    for (tl, st, cnt) in segments:
                        h = cnt // 2
                        if h:
                            nc.vector.tensor_tensor(
                                out=dest[:, pos:pos + h, :],
                                in0=tl[:, st:st + h, :],
                                in1=tl[:, st + h:st + 2 * h, :], op=MAX)
                            pos += h
                        if cnt % 2:
                            singles.append((tl, st + 2 * h))
                    while len(singles) >= 2:
                        (t0, s0), (t1, s1) = singles.pop(), singles.pop()
                        nc.vector.tensor_tensor(
                            out=dest[:, pos:pos + 1, :],
                            in0=t0[:, s0:s0 + 1, :],
                            in1=t1[:, s1:s1 + 1, :], op=MAX)
                        pos += 1
                    segments = [(dest, 0, pos)] + \
                        [(tl, st, 1) for (tl, st) in singles]
                    total = sum(c for (_, _, c) in segments)
                    level += 1
                mxt, mxs, _ = segments[0]
                mx_ap = mxt[:, mxs:mxs + 1, :]

                # ---- one-hot via is_equal against broadcast mx ----
                out_f = outp.tile([128, nreal, WMAX], f32, tag="out_t")
                out_t = out_f[:, :, 0:G]
                if nk:
                    nc.vector.tensor_tensor(
                        out=out_t[:, 0:nk, :], in0=kf[:],
                        in1=_bcast_mid(mx_ap, nk), op=EQ)
                if nu:
                    nc.vector.tensor_tensor(
                        out=out_t[:, nk:nreal, :], in0=in_t[:, nk:nreal, :],
                        in1=_bcast_mid(mx_ap, nu), op=EQ)

                # ---- tie detector: per-partition popcount of the one-hot ----
                if DET_ON_ACT:
                    dummy = work.tile([128, nreal, WMAX], f16, tag="dummy")
                    nc.scalar.activation(
                        dummy[:, :, 0:G], out_t,
                        mybir.ActivationFunctionType.Identity,
                        accum_out=acc[:, t:t + 1])
                else:
                    nc.vector.tensor_reduce(
                        out=acc[:, t:t + 1], in_=out_t[:],
                        axis=mybir.AxisListType.XY, op=mybir.AluOpType.add)

                nc.sync.dma_start(
                    out_dram[:, :, sl].rearrange("c p g -> p c g"), out_t[:])

            nc.sync.dma_start(acc_dram[:], acc[:])

    orig = nc.to_json_bytes
    nc.to_json_bytes = lambda: _split_excess_waits(orig())
    _cache[key] = nc
    return nc


def kernel(masks, slot_features, pad_left, pad_top):
    from concourse.bass_utils import run_bass_kernel_spmd

    masks = np.asarray(masks, np.float32)
    slot_features = np.asarray(slot_features, np.float32)
    pl = [int(v) for v in np.asarray(pad_left)]
    pt = [int(v) for v in np.asarray(pad_top)]

    remap = _compute_remap(masks, slot_features, pl, pt)
    slot_order, real_channels, nk, round_sizes = _slot_layout(remap)
    nreal = len(real_channels)

    nc = _build_program(nk, round_sizes, nreal)

    m0 = masks[0][slot_order]           # [32, H, W] slot-ordered
    in_maps = []
    for i in range(N_CORES):
        slab = np.ascontiguousarray(
            m0[:, i * ROWS_PER_CORE:(i + 1) * ROWS_PER_CORE, :])
        in_maps.append({"masks": slab})

    res = run_bass_kernel_spmd(nc, in_maps, core_ids=list(range(N_CORES)))

    out = np.zeros((1, C, IMG_H, IMG_W), np.float32)
    real_idx = np.asarray(real_channels)
    flagged_rows = []
    for i, r in enumerate(res.results):
        out[0, real_idx, i * ROWS_PER_CORE:(i + 1) * ROWS_PER_CORE, :] = \
            r["out"]
        acc = r["acc"]                  # [128, NTILES]
        bad = np.where(acc.sum(axis=1) != float(IMG_W))[0]
        for p in bad:
            flagged_rows.append(i * ROWS_PER_CORE + int(p))

    # exact host patch of rows containing argmax ties (typically <= 3 rows)
    for y in flagged_rows:
        w = np.argmax(masks[0, :, y, :], axis=0)          # first max, like jnp
        out[0, :, y, :] = 0.0
        out[0, remap[w], y, np.arange(IMG_W)] = 1.0
    return out


# revision 10
# speedup vs baseline: 1.1010x; 1.0195x over previous
"""nn_MergeWindows — Trainium2 Bass kernel (8 NeuronCores, SPMD over image rows).

The reference computes out[b,c,y,x] = 1.0 iff remap[argmax_d masks[b,d,y,x]] == c,
where remap: [32]->[32] merges channels according to a sequential scan over
window-adjacency candidate pairs.  remap depends only on tiny metadata (cosine
sims of the [4,7,64] slot features and per-channel edge-touch bits along the
four window boundary strips) and is computed on the host in microseconds.

Device strategy (per core: 128 image rows, [32, 128, 1024] slab, slot-ordered):

  Channels are permuted host-side into slots [keepers | unmerged | rems], where
  a "keeper" receives >=1 merged channel.  Since out[c] = 1 iff the max over
  c's merge group equals the pixel max, the device:
    1. max-accumulates each rem slot into its keeper slot (merge "rounds" —
       one strided tensor_tensor max per round over all keepers needing it),
    2. tree-maxes the NREAL real slots -> per-pixel max mx,
    3. emits the one-hot with two is_equal ops against broadcast mx,
    4. detects argmax TIES (pixels where >=2 real slots equal mx) via an ACT
       Identity op with accum_out: per-partition sum over the one-hot == G
       unless a tie occurred.  Flagged rows (a handful per image at most) are
       recomputed exactly on the host, making the kernel exact for all inputs.
  Merged-away channels are identically zero: the device only writes the NREAL
  live channels (9.5 MiB instead of 16 MiB per core) and the host zero-fills
  the rest.
"""

import json

import numpy as np

N_WINDOWS = 4
WIN_H = WIN_W = 512
IMG_H = IMG_W = 1024
C = 32
MPW = C // N_WINDOWS
SIM_THRESH = 0.1

N_CORES = 8
ROWS_PER_CORE = IMG_H // N_CORES  # 128
# column-tile widths: narrow first/last tiles shorten the pipeline ramp
# (compute starts after tile 0 lands) and the drain tail
WIDTHS = (256, 256, 256, 192, 64)
NTILES = len(WIDTHS)
assert sum(WIDTHS) == IMG_W

MERGE_INPLACE = True     # accumulate rounds in place on the keeper tile
DET_ON_ACT = True        # tie detector via scalar-engine accum_out

_cache = {}


# --------------------------------------------------------------------------
# host-side merge decision (mirrors reference._merge_windows metadata math)
# --------------------------------------------------------------------------
def _compute_remap(masks, slot_features, pl, pt):
    B, Ch, H, W = masks.shape
    mpw = Ch // N_WINDOWS
    ranges = [(i * mpw, (i + 1) * mpw) for i in range(N_WINDOWS)]

    adjacency = []
    for i in range(N_WINDOWS):
        for j in range(i + 1, N_WINDOWS):
            if pt[i] == pt[j] and abs(pl[i] - pl[j]) == WIN_W:
                adjacency.append((i, j, True) if pl[i] < pl[j] else (j, i, True))
            if pl[i] == pl[j] and abs(pt[i] - pt[j]) == WIN_H:
                adjacency.append((i, j, False) if pt[i] < pt[j] else (j, i, False))

    edge_l = np.zeros(Ch, bool)
    edge_r = np.zeros(Ch, bool)
    edge_t = np.zeros(Ch, bool)
    edge_b = np.zeros(Ch, bool)
    m0 = masks[0]
    for wi, (s, e) in enumerate(ranges):
        ys, ye = max(pt[wi], 0), min(pt[wi] + WIN_H, H)
        xs, xe = max(pl[wi], 0), min(pl[wi] + WIN_W, W)
        if ys >= ye or xs >= xe:
            continue
        ids_l = np.argmax(m0[:, ys:ye, xs], axis=0)
        ids_r = np.argmax(m0[:, ys:ye, xe - 1], axis=0)
        ids_t = np.argmax(m0[:, ys, xs:xe], axis=0)
        ids_b = np.argmax(m0[:, ye - 1, xs:xe], axis=0)
        for k in range(s, e):
            edge_l[k] = np.any(ids_l == k)
            edge_r[k] = np.any(ids_r == k)
            edge_t[k] = np.any(ids_t == k)
            edge_b[k] = np.any(ids_b == k)

    ci_l, cj_l, wi_l, wj_l, hz_l = [], [], [], [], []
    for wi, wj, horiz in adjacency:
        si, ei = ranges[wi]
        sj, ej = ranges[wj]
        for ci in range(si + 1, ei):
            for cj in range(sj + 1, ej):
                ci_l.append(ci)
                cj_l.append(cj)
                wi_l.append(wi)
                wj_l.append(wj)
                hz_l.append(horiz)

    target = np.arange(Ch)
    if not ci_l:
        return target

    sf = np.asarray(slot_features, np.float32)
    sf_n = sf / (np.linalg.norm(sf, axis=-1, keepdims=True) + np.float32(1e-8))
    ci_a = np.array(ci_l)
    cj_a = np.array(cj_l)
    rel_i = ci_a % mpw - 1
    rel_j = cj_a % mpw - 1
    fi = sf_n[np.array(wi_l), rel_i]
    fj = sf_n[np.array(wj_l), rel_j]
    sims = np.sum(fi * fj, axis=-1)
    hz = np.array(hz_l)
    edge_ok = np.where(hz, edge_r[ci_a] & edge_l[cj_a], edge_b[ci_a] & edge_t[cj_a])
    passing = edge_ok & (sims > np.float32(SIM_THRESH))

    merged = np.zeros(Ch, bool)
    for ci, cj, ok in zip(ci_l, cj_l, passing):
        if ok and not merged[ci] and not merged[cj]:
            keep, rem = min(ci, cj), max(ci, cj)
            target[target == rem] = keep
            merged[rem] = True
    return target


def _slot_layout(remap):
    """Channel->slot permutation and merge-round structure from remap.

    Returns (slot_order, real_channels, nk, round_sizes) where slot_order[s]
    is the original channel loaded into slot s; slots [0:nk] keepers sorted by
    descending rem count, [nk:nreal] unmerged, [nreal:32] rems grouped so that
    round r's rems (the r-th rem of each keeper that has one) are contiguous
    and aligned with keeper slots [0:round_sizes[r]].
    """
    rems_of = {}
    for c in range(C):
        k = int(remap[c])
        if k != c:
            rems_of.setdefault(k, []).append(c)
    keepers = sorted(rems_of, key=lambda k: (-len(rems_of[k]), k))
    unmerged = [c for c in range(C)
                if int(remap[c]) == c and c not in rems_of]
    nk = len(keepers)
    nreal = nk + len(unmerged)
    max_rounds = max((len(v) for v in rems_of.values()), default=0)
    round_slots = []
    round_sizes = []
    for r in range(max_rounds):
        rs = [rems_of[k][r] for k in keepers if len(rems_of[k]) > r]
        round_sizes.append(len(rs))
        round_slots.extend(rs)
    slot_order = keepers + unmerged + round_slots
    assert len(slot_order) == C
    return slot_order, keepers + unmerged, nk, tuple(round_sizes)


# --------------------------------------------------------------------------
# wait-split post-pass: the pinned neuronxcc allows only ONE sync wait per
# instruction; hoist extras onto preceding same-engine EventSemaphore insts.
# --------------------------------------------------------------------------
def _split_excess_waits(bir_json_bytes, limit=1):
    j = json.loads(bir_json_bytes)
    counter = [0]
    for fn in j.get("functions", []):
        for bb in fn.get("blocks", []):
            new_insts = []
            for inst in bb.get("instructions", []):
                si = inst.get("sync_info") or {}
                waits = si.get("on_wait") or []
                if len(waits) > limit:
                    extra = waits[: len(waits) - limit]
                    si["on_wait"] = waits[len(waits) - limit:]
                    inst["sync_info"] = si
                    for i in range(0, len(extra), limit):
                        counter[0] += 1
                        new_insts.append({
                            "engine": inst["engine"],
                            "ins": [],
                            "name": f"{inst['name']}_hoistw{counter[0]}",
                            "opcode": "EventSemaphore",
                            "outs": [],
                            "sync_info": {"on_update": [],
                                          "on_wait": extra[i: i + limit]},
                        })
                new_insts.append(inst)
            bb["instructions"] = new_insts
    return json.dumps(j).encode()


def _bcast_mid(ap, n):
    """Broadcast a [128, 1, G]-ish AP over n slots (middle dim, stride 0)."""
    import concourse.bass as bass
    return bass.AP(tensor=ap.tensor, offset=ap.offset,
                   ap=[ap.ap[0], [0, n], ap.ap[-1]])


def _build_program(nk, round_sizes, nreal):
    key = ("prog", nk, round_sizes, nreal)
    if key in _cache:
        return _cache[key]

    import concourse.bass as bass
    import concourse.tile as tile
    from concourse import mybir

    f32 = mybir.dt.float32
    f16 = mybir.dt.float16
    MAX = mybir.AluOpType.max
    EQ = mybir.AluOpType.is_equal

    nu = nreal - nk          # unmerged slot count

    nc = bass.Bass()
    masks_in = nc.dram_tensor("masks", [C, ROWS_PER_CORE, IMG_W], f32,
                              kind="ExternalInput")
    out_dram = nc.dram_tensor("out", [nreal, ROWS_PER_CORE, IMG_W], f32,
                              kind="ExternalOutput")
    acc_dram = nc.dram_tensor("acc", [128, NTILES], f32,
                              kind="ExternalOutput")

    with tile.TileContext(nc) as tc:
        with (
            tc.tile_pool(name="inp", bufs=3) as inp,
            tc.tile_pool(name="outp", bufs=2) as outp,
            tc.tile_pool(name="work", bufs=1) as work,
            tc.tile_pool(name="singles", bufs=1) as singles,
        ):
            acc = singles.tile([128, NTILES], f32)

            WMAX = max(WIDTHS)
            col = 0
            for t in range(NTILES):
                G = WIDTHS[t]
                sl = slice(col, col + G)
                col += G
                in_f = inp.tile([128, C, WMAX], f32, tag="in_t")
                in_t = in_f[:, :, 0:G]
                nc.sync.dma_start(
                    in_t, masks_in[:, :, sl].rearrange("d p g -> p d g"))

                # ---- merge rounds: accumulate rems into keeper slots ----
                if nk:
                    kf_f = work.tile([128, nk, WMAX], f32, tag="kf")
                    kf = kf_f[:, :, 0:G]
                    off = nreal
                    for r, nr in enumerate(round_sizes):
                        if r == 0:
                            nc.vector.tensor_tensor(
                                out=kf[:], in0=in_t[:, 0:nk, :],
                                in1=in_t[:, off:off + nr, :], op=MAX)
                        else:
                            nc.vector.tensor_tensor(
                                out=kf[:, 0:nr, :], in0=kf[:, 0:nr, :],
                                in1=in_t[:, off:off + nr, :], op=MAX)
                        off += nr

                # ---- tree max over the nreal real slots -> mx [128,1,G] ----
                # segments: (tile, start, count) pieces holding real values
                segments = []
                if nk:
                    segments.append((kf, 0, nk))
                if nu:
                    segments.append((in_t, nk, nu))
                level = 0
                total = sum(c for (_, _, c) in segments)
                while total > 1:
                    dest_f = work.tile([128, (total + 1) // 2, WMAX], f32,
                                       tag=f"tree{level}")
                    dest = dest_f[:, :, 0:G]
                    pos = 0
                    singles = []
                    for (tl, st, cnt) in segments:
                        h = cnt // 2
                        if h:
                            nc.vector.tensor_tensor(
                                out=dest[:, pos:pos + h, :],
                                in0=tl[:, st:st + h, :],
                                in1=tl[:, st + h:st + 2 * h, :], op=MAX)
                            pos += h
                        if cnt % 2:
                            singles.append((tl, st + 2 * h))
                    while len(singles) >= 2:
                        (t0, s0), (t1, s1) = singles.pop(), singles.pop()
                        nc.vector.tensor_tensor(
                            out=dest[:, pos:pos + 1, :],
                            in0=t0[:, s0:s0 + 1, :],
                            in1=t1[:, s1:s1 + 1, :], op=MAX)
                        pos += 1
                    segments = [(dest, 0, pos)] + \
                        [(tl, st, 1) for (tl, st) in singles]
                    total = sum(c for (_, _, c) in segments)
                    level += 1
                mxt, mxs, _ = segments[0]
                mx_ap = mxt[:, mxs:mxs + 1, :]

                # ---- one-hot via is_equal against broadcast mx ----
                out_f = outp.tile([128, nreal, WMAX], f32, tag="out_t")
                out_t = out_f[:, :, 0:G]
                if nk:
                    nc.vector.tensor_tensor(
                        out=out_t[:, 0:nk, :], in0=kf[:],
                        in1=_bcast_mid(mx_ap, nk), op=EQ)
                if nu:
                    nc.vector.tensor_tensor(
                        out=out_t[:, nk:nreal, :], in0=in_t[:, nk:nreal, :],
                        in1=_bcast_mid(mx_ap, nu), op=EQ)

                # ---- tie detector: per-partition popcount of the one-hot ----
                if DET_ON_ACT:
                    dummy = work.tile([128, nreal, WMAX], f16, tag="dummy")
                    nc.scalar.activation(
                        dummy[:, :, 0:G], out_t,
                        mybir.ActivationFunctionType.Identity,
                        accum_out=acc[:, t:t + 1])
                else:
                    nc.vector.tensor_reduce(
                        out=acc[:, t:t + 1], in_=out_t[:],
                        axis=mybir.AxisListType.XY, op=mybir.AluOpType.add)

                nc.sync.dma_start(
                    out_dram[:, :, sl].rearrange("c p g -> p c g"), out_t[:])

            nc.sync.dma_start(acc_dram[:], acc[:])

    orig = nc.to_json_bytes
    nc.to_json_bytes = lambda: _split_excess_waits(orig())
    _cache[key] = nc
    return nc


def kernel(masks, slot_features, pad_left, pad_top):
    from concourse.bass_utils import run_bass_kernel_spmd

    masks = np.asarray(masks, np.float32)
    slot_features = np.asarray(slot_features, np.float32)
    pl = [int(v) for v in np.asarray(pad_left)]
    pt = [int(v) for v in np.asarray(pad_top)]

    remap = _compute_remap(masks, slot_features, pl, pt)
    slot_order, real_channels, nk, round_sizes = _slot_layout(remap)
    nreal = len(real_channels)

    nc = _build_program(nk, round_sizes, nreal)

    m0 = masks[0][slot_order]           # [32, H, W] slot-ordered
    in_maps = []
    for i in range(N_CORES):
        slab = np.ascontiguousarray(
            m0[:, i * ROWS_PER_CORE:(i + 1) * ROWS_PER_CORE, :])
        in_maps.append({"masks": slab})

    res = run_bass_kernel_spmd(nc, in_maps, core_ids=list(range(N_CORES)))

    out = np.zeros((1, C, IMG_H, IMG_W), np.float32)
    real_idx = np.asarray(real_channels)
    flagged_rows = []
    for i, r in enumerate(res.results):
        out[0, real_idx, i * ROWS_PER_CORE:(i + 1) * ROWS_PER_CORE, :] = \
            r["out"]
        acc = r["acc"]                  # [128, NTILES]
        bad = np.where(acc.sum(axis=1) != float(IMG_W))[0]
        for p in bad:
            flagged_rows.append(i * ROWS_PER_CORE + int(p))

    # exact host patch of rows containing argmax ties (typically <= 3 rows)
    for y in flagged_rows:
        w = np.argmax(masks[0, :, y, :], axis=0)          # first max, like jnp
        out[0, :, y, :] = 0.0
        out[0, remap[w], y, np.arange(IMG_W)] = 1.0
    return out


# revision 11
# speedup vs baseline: 1.1094x; 1.0076x over previous
"""nn_MergeWindows — Trainium2 Bass kernel (8 NeuronCores, SPMD over image rows).

The reference computes out[b,c,y,x] = 1.0 iff remap[argmax_d masks[b,d,y,x]] == c,
where remap: [32]->[32] merges channels according to a sequential scan over
window-adjacency candidate pairs.  remap depends only on tiny metadata (cosine
sims of the [4,7,64] slot features and per-channel edge-touch bits along the
four window boundary strips) and is computed on the host in microseconds.

Device strategy (per core: 128 image rows, [32, 128, 1024] slab, slot-ordered):

  Channels are permuted host-side into slots [keepers | unmerged | rems], where
  a "keeper" receives >=1 merged channel.  Since out[c] = 1 iff the max over
  c's merge group equals the pixel max, the device:
    1. max-accumulates each rem slot into its keeper slot (merge "rounds" —
       one strided tensor_tensor max per round over all keepers needing it),
    2. tree-maxes the NREAL real slots -> per-pixel max mx,
    3. emits the one-hot with two is_equal ops against broadcast mx,
    4. detects argmax TIES (pixels where >=2 real slots equal mx) via an ACT
       Identity op with accum_out: per-partition sum over the one-hot == G
       unless a tie occurred.  Flagged rows (a handful per image at most) are
       recomputed exactly on the host, making the kernel exact for all inputs.
  Merged-away channels are identically zero: the device only writes the NREAL
  live channels (9.5 MiB instead of 16 MiB per core) and the host zero-fills
  the rest.
"""

import json

import numpy as np

N_WINDOWS = 4
WIN_H = WIN_W = 512
IMG_H = IMG_W = 1024
C = 32
MPW = C // N_WINDOWS
SIM_THRESH = 0.1

N_CORES = 8
ROWS_PER_CORE = IMG_H // N_CORES  # 128
# column-tile widths: narrow first/last tiles shorten the pipeline ramp
# (compute starts after tile 0 lands) and the drain tail
WIDTHS = (256, 256, 256, 192, 64)
NTILES = len(WIDTHS)
assert sum(WIDTHS) == IMG_W

MERGE_INPLACE = True     # accumulate rounds in place on the keeper tile
DET_ON_ACT = True        # tie detector via scalar-engine accum_out

_cache = {}


# --------------------------------------------------------------------------
# host-side merge decision (mirrors reference._merge_windows metadata math)
# --------------------------------------------------------------------------
def _compute_remap(masks, slot_features, pl, pt):
    B, Ch, H, W = masks.shape
    mpw = Ch // N_WINDOWS
    ranges = [(i * mpw, (i + 1) * mpw) for i in range(N_WINDOWS)]

    adjacency = []
    for i in range(N_WINDOWS):
        for j in range(i + 1, N_WINDOWS):
            if pt[i] == pt[j] and abs(pl[i] - pl[j]) == WIN_W:
                adjacency.append((i, j, True) if pl[i] < pl[j] else (j, i, True))
            if pl[i] == pl[j] and abs(pt[i] - pt[j]) == WIN_H:
                adjacency.append((i, j, False) if pt[i] < pt[j] else (j, i, False))

    edge_l = np.zeros(Ch, bool)
    edge_r = np.zeros(Ch, bool)
    edge_t = np.zeros(Ch, bool)
    edge_b = np.zeros(Ch, bool)
    m0 = masks[0]
    for wi, (s, e) in enumerate(ranges):
        ys, ye = max(pt[wi], 0), min(pt[wi] + WIN_H, H)
        xs, xe = max(pl[wi], 0), min(pl[wi] + WIN_W, W)
        if ys >= ye or xs >= xe:
            continue
        ids_l = np.argmax(m0[:, ys:ye, xs], axis=0)
        ids_r = np.argmax(m0[:, ys:ye, xe - 1], axis=0)
        ids_t = np.argmax(m0[:, ys, xs:xe], axis=0)
        ids_b = np.argmax(m0[:, ye - 1, xs:xe], axis=0)
        for k in range(s, e):
            edge_l[k] = np.any(ids_l == k)
            edge_r[k] = np.any(ids_r == k)
            edge_t[k] = np.any(ids_t == k)
            edge_b[k] = np.any(ids_b == k)

    ci_l, cj_l, wi_l, wj_l, hz_l = [], [], [], [], []
    for wi, wj, horiz in adjacency:
        si, ei = ranges[wi]
        sj, ej = ranges[wj]
        for ci in range(si + 1, ei):
            for cj in range(sj + 1, ej):
                ci_l.append(ci)
                cj_l.append(cj)
                wi_l.append(wi)
                wj_l.append(wj)
                hz_l.append(horiz)

    target = np.arange(Ch)
    if not ci_l:
        return target

    sf = np.asarray(slot_features, np.float32)
    sf_n = sf / (np.linalg.norm(sf, axis=-1, keepdims=True) + np.float32(1e-8))
    ci_a = np.array(ci_l)
    cj_a = np.array(cj_l)
    rel_i = ci_a % mpw - 1
    rel_j = cj_a % mpw - 1
    fi = sf_n[np.array(wi_l), rel_i]
    fj = sf_n[np.array(wj_l), rel_j]
    sims = np.sum(fi * fj, axis=-1)
    hz = np.array(hz_l)
    edge_ok = np.where(hz, edge_r[ci_a] & edge_l[cj_a], edge_b[ci_a] & edge_t[cj_a])
    passing = edge_ok & (sims > np.float32(SIM_THRESH))

    merged = np.zeros(Ch, bool)
    for ci, cj, ok in zip(ci_l, cj_l, passing):
        if ok and not merged[ci] and not merged[cj]:
            keep, rem = min(ci, cj), max(ci, cj)
            target[target == rem] = keep
            merged[rem] = True
    return target


def _slot_layout(remap):
    """Channel->slot permutation and merge-round structure from remap.

    Returns (slot_order, real_channels, nk, round_sizes) where slot_order[s]
    is the original channel loaded into slot s; slots [0:nk] keepers sorted by
    descending rem count, [nk:nreal] unmerged, [nreal:32] rems grouped so that
    round r's rems (the r-th rem of each keeper that has one) are contiguous
    and aligned with keeper slots [0:round_sizes[r]].
    """
    rems_of = {}
    for c in range(C):
        k = int(remap[c])
        if k != c:
            rems_of.setdefault(k, []).append(c)
    keepers = sorted(rems_of, key=lambda k: (-len(rems_of[k]), k))
    unmerged = [c for c in range(C)
                if int(remap[c]) == c and c not in rems_of]
    nk = len(keepers)
    nreal = nk + len(unmerged)
    max_rounds = max((len(v) for v in rems_of.values()), default=0)
    round_slots = []
    round_sizes = []
    for r in range(max_rounds):
        rs = [rems_of[k][r] for k in keepers if len(rems_of[k]) > r]
        round_sizes.append(len(rs))
        round_slots.extend(rs)
    slot_order = keepers + unmerged + round_slots
    assert len(slot_order) == C
    return slot_order, keepers + unmerged, nk, tuple(round_sizes)


# --------------------------------------------------------------------------
# wait-split post-pass: the pinned neuronxcc allows only ONE sync wait per
# instruction; hoist extras onto preceding same-engine EventSemaphore insts.
# --------------------------------------------------------------------------
def _split_excess_waits(bir_json_bytes, limit=1):
    j = json.loads(bir_json_bytes)
    counter = [0]
    for fn in j.get("functions", []):
        for bb in fn.get("blocks", []):
            new_insts = []
            for inst in bb.get("instructions", []):
                si = inst.get("sync_info") or {}
                waits = si.get("on_wait") or []
                if len(waits) > limit:
                    extra = waits[: len(waits) - limit]
                    si["on_wait"] = waits[len(waits) - limit:]
                    inst["sync_info"] = si
                    for i in range(0, len(extra), limit):
                        counter[0] += 1
                        new_insts.append({
                            "engine": inst["engine"],
                            "ins": [],
                            "name": f"{inst['name']}_hoistw{counter[0]}",
                            "opcode": "EventSemaphore",
                            "outs": [],
                            "sync_info": {"on_update": [],
                                          "on_wait": extra[i: i + limit]},
                        })
                new_insts.append(inst)
            bb["instructions"] = new_insts
    return json.dumps(j).encode()


def _bcast_mid(ap, n):
    """Broadcast a [128, 1, G]-ish AP over n slots (middle dim, stride 0)."""
    import concourse.bass as bass
    return bass.AP(tensor=ap.tensor, offset=ap.offset,
                   ap=[ap.ap[0], [0, n], ap.ap[-1]])


def _build_program(nk, round_sizes, nreal):
    key = ("prog", nk, round_sizes, nreal)
    if key in _cache:
        return _cache[key]

    import concourse.bass as bass
    import concourse.tile as tile
    from concourse import mybir

    f32 = mybir.dt.float32
    f16 = mybir.dt.float16
    MAX = mybir.AluOpType.max
    EQ = mybir.AluOpType.is_equal

    nu = nreal - nk          # unmerged slot count

    nc = bass.Bass()
    masks_in = nc.dram_tensor("masks", [C, ROWS_PER_CORE, IMG_W], f32,
                              kind="ExternalInput")
    out_dram = nc.dram_tensor("out", [nreal, ROWS_PER_CORE, IMG_W], f32,
                              kind="ExternalOutput")
    acc_dram = nc.dram_tensor("acc", [128, NTILES], f32,
                              kind="ExternalOutput")

    with tile.TileContext(nc) as tc:
        with (
            tc.tile_pool(name="inp", bufs=4) as inp,
            tc.tile_pool(name="outp", bufs=2) as outp,
            tc.tile_pool(name="work", bufs=1) as work,
            tc.tile_pool(name="singles", bufs=1) as singles,
        ):
            acc = singles.tile([128, NTILES], f32)

            WMAX = max(WIDTHS)
            col = 0
            for t in range(NTILES):
                G = WIDTHS[t]
                sl = slice(col, col + G)
                col += G
                in_f = inp.tile([128, C, WMAX], f32, tag="in_t")
                in_t = in_f[:, :, 0:G]
                nc.sync.dma_start(
                    in_t, masks_in[:, :, sl].rearrange("d p g -> p d g"))

                # ---- merge rounds: accumulate rems into keeper slots ----
                if nk:
                    kf_f = work.tile([128, nk, WMAX], f32, tag="kf")
                    kf = kf_f[:, :, 0:G]
                    off = nreal
                    for r, nr in enumerate(round_sizes):
                        if r == 0:
                            nc.vector.tensor_tensor(
                                out=kf[:], in0=in_t[:, 0:nk, :],
                                in1=in_t[:, off:off + nr, :], op=MAX)
                        else:
                            nc.vector.tensor_tensor(
                                out=kf[:, 0:nr, :], in0=kf[:, 0:nr, :],
                                in1=in_t[:, off:off + nr, :], op=MAX)
                        off += nr

                # ---- tree max over the nreal real slots -> mx [128,1,G] ----
                # segments: (tile, start, count) pieces holding real values
                segments = []
                if nk:
                    segments.append((kf, 0, nk))
                if nu:
                    segments.append((in_t, nk, nu))
                level = 0
                total = sum(c for (_, _, c) in segments)
                while total > 1:
                    dest_f = work.tile([128, (total + 1) // 2, WMAX], f32,
                                       tag=f"tree{level}")
                    dest = dest_f[:, :, 0:G]
                    pos = 0
                    singles = []
                    for (tl, st, cnt) in segments:
                        h = cnt // 2
                        if h:
                            nc.vector.tensor_tensor(
                                out=dest[:, pos:pos + h, :],
                                in0=tl[:, st:st + h, :],
                                in1=tl[:, st + h:st + 2 * h, :], op=MAX)
                            pos += h
                        if cnt % 2:
                            singles.append((tl, st + 2 * h))
                    while len(singles) >= 2:
                        (t0, s0), (t1, s1) = singles.pop(), singles.pop()
                        nc.vector.tensor_tensor(
                            out=dest[:, pos:pos + 1, :],
                            in0=t0[:, s0:s0 + 1, :],
                            in1=t1[:, s1:s1 + 1, :], op=MAX)
                        pos += 1
                    segments = [(dest, 0, pos)] + \
                        [(tl, st, 1) for (tl, st) in singles]
                    total = sum(c for (_, _, c) in segments)
                    level += 1
                mxt, mxs, _ = segments[0]
                mx_ap = mxt[:, mxs:mxs + 1, :]

                # ---- one-hot via is_equal against broadcast mx ----
                out_f = outp.tile([128, nreal, WMAX], f32, tag="out_t")
                out_t = out_f[:, :, 0:G]
                if nk:
                    nc.vector.tensor_tensor(
                        out=out_t[:, 0:nk, :], in0=kf[:],
                        in1=_bcast_mid(mx_ap, nk), op=EQ)
                if nu:
                    nc.vector.tensor_tensor(
                        out=out_t[:, nk:nreal, :], in0=in_t[:, nk:nreal, :],
                        in1=_bcast_mid(mx_ap, nu), op=EQ)

                # ---- tie detector: per-partition popcount of the one-hot ----
                if DET_ON_ACT:
                    dummy = work.tile([128, nreal, WMAX], f16, tag="dummy")
                    nc.scalar.activation(
                        dummy[:, :, 0:G], out_t,
                        mybir.ActivationFunctionType.Identity,
                        accum_out=acc[:, t:t + 1])
                else:
                    nc.vector.tensor_reduce(
                        out=acc[:, t:t + 1], in_=out_t[:],
                        axis=mybir.AxisListType.XY, op=mybir.AluOpType.add)

                nc.sync.dma_start(
                    out_dram[:, :, sl].rearrange("c p g -> p c g"), out_t[:])

            nc.sync.dma_start(acc_dram[:], acc[:])

    orig = nc.to_json_bytes
    nc.to_json_bytes = lambda: _split_excess_waits(orig())
    _cache[key] = nc
    return nc


def kernel(masks, slot_features, pad_left, pad_top):
    from concourse.bass_utils import run_bass_kernel_spmd

    masks = np.asarray(masks, np.float32)
    slot_features = np.asarray(slot_features, np.float32)
    pl = [int(v) for v in np.asarray(pad_left)]
    pt = [int(v) for v in np.asarray(pad_top)]

    remap = _compute_remap(masks, slot_features, pl, pt)
    slot_order, real_channels, nk, round_sizes = _slot_layout(remap)
    nreal = len(real_channels)

    nc = _build_program(nk, round_sizes, nreal)

    m0 = masks[0][slot_order]           # [32, H, W] slot-ordered
    in_maps = []
    for i in range(N_CORES):
        slab = np.ascontiguousarray(
            m0[:, i * ROWS_PER_CORE:(i + 1) * ROWS_PER_CORE, :])
        in_maps.append({"masks": slab})

    res = run_bass_kernel_spmd(nc, in_maps, core_ids=list(range(N_CORES)))

    out = np.zeros((1, C, IMG_H, IMG_W), np.float32)
    real_idx = np.asarray(real_channels)
    flagged_rows = []
    for i, r in enumerate(res.results):
        out[0, real_idx, i * ROWS_PER_CORE:(i + 1) * ROWS_PER_CORE, :] = \
            r["out"]
        acc = r["acc"]                  # [128, NTILES]
        bad = np.where(acc.sum(axis=1) != float(IMG_W))[0]
        for p in bad:
            flagged_rows.append(i * ROWS_PER_CORE + int(p))

    # exact host patch of rows containing argmax ties (typically <= 3 rows)
    for y in flagged_rows:
        w = np.argmax(masks[0, :, y, :], axis=0)          # first max, like jnp
        out[0, :, y, :] = 0.0
        out[0, remap[w], y, np.arange(IMG_W)] = 1.0
    return out
